# revision 1
# baseline (speedup 1.0000x reference)
"""Trainium2 Bass kernel for CombinedMSESSIMLoss (MSE + SSIM + EPI + PSNR).

Contract: kernel(output, target) -> np.float32 scalar loss, computed on 8
NeuronCores, data-parallel over the batch dim (65536 images of 28x28).

Structure:
  pass 1 (per core): max(target), max(-target) over the 8192-image shard.
  host: combine -> data_range -> C1, C2 ssim constants.
  pass 2 (per core): all remaining partial sums in one streamed kernel:
    - transposes each 128-image tile to pixel-major via the PE,
    - SSIM 11x11 valid gaussian filtering as dense [112,108] PE matmuls over
      {x+y, x-y, x^2+y^2, 2xy} (error-compensated f32r weights),
    - ssim rational map on DVE/ACT/GPSIMD with per-tile accumulators,
    - per-image sobel d-maps as banded PE matmuls (exact integer weights),
    - batch-axis [1,2,1] smoothing handled algebraically: sum S_x S_y =
      sum_{|b-b'|<=2} M[b,b'] dx[b].dy[b'] with M = A A^T pentadiagonal,
      evaluated via PE gram matrices + weighted reductions; tile/core
      boundary pairs via stashed edge columns + one cross-gram at the end.
  host: assemble loss in float64 (log10, sqrt, divisions).
"""
import json

import numpy as np

import concourse.bass as bass
import concourse.tile as tile
from concourse import mybir
from concourse.bass_utils import run_bass_kernel_spmd

F32 = mybir.dt.float32
F32R = mybir.dt.float32r
ALU = mybir.AluOpType
ACTF = mybir.ActivationFunctionType
AX = mybir.AxisListType

H = W = 28
PIX = H * W
NCHUNK = 7
CK = 112
MOUT = 324
MCH = 3
MK = 108
WIN, SIGMA, K1, K2 = 11, 1.5, 0.01, 0.03
OW = 18
RS2 = float(1.0 / np.sqrt(2.0))

B_GLOB = 65536
N_CORES = 8
B_LOC = B_GLOB // N_CORES     # 8192
T_TILES = B_LOC // 128        # 64

MSE_W, SSIM_W, EPI_W, PSNR_W = 1.0, 0.5, 0.1, 0.01


# ---------------------------------------------------------------- walrus fix
# This walrus build rejects >1 sync-wait per instruction; split extra waits
# onto single-wait NoOps ahead of the instruction.
_orig_to_json_bytes = bass.Bass.to_json_bytes


def _split_waits(obj):
    if isinstance(obj, dict):
        ilist = obj.get("instructions")
        if isinstance(ilist, list):
            newlist = []
            for ins in ilist:
                try:
                    w = ins.get("sync_info", {}).get("on_wait", [])
                except AttributeError:
                    w = []
                if isinstance(w, list) and len(w) > 1:
                    for k, wt in enumerate(w[:-1]):
                        newlist.append({
                            "debug": ins.get("debug", 0),
                            "engine": ins["engine"],
                            "ins": [], "outs": [],
                            "name": str(ins["name"]) + f"_wsplit{k}",
                            "opcode": "NoOp",
                            "sync_info": {"on_update": [], "on_wait": [wt]},
                        })
                    ins["sync_info"]["on_wait"] = [w[-1]]
                newlist.append(ins)
            obj["instructions"] = newlist
        for v in obj.values():
            _split_waits(v)
    elif isinstance(obj, list):
        for v in obj:
            _split_waits(v)


def _patched_to_json_bytes(self, *a, **k):
    data = json.loads(_orig_to_json_bytes(self, *a, **k))
    _split_waits(data)
    return json.dumps(data).encode()


bass.Bass.to_json_bytes = _patched_to_json_bytes


# ----------------------------------------------------------- const builders

def _gauss1d():
    c = np.arange(WIN, dtype=np.float64) - WIN // 2
    g = np.exp(-(c ** 2) / (2.0 * SIGMA ** 2))
    return g / g.sum()


def _build_L():
    g = _gauss1d()
    L = np.zeros((PIX, MOUT), dtype=np.float64)
    for hp in range(OW):
        for wp in range(OW):
            q = hp * OW + wp
            for kh in range(WIN):
                for kw in range(WIN):
                    L[(hp + kh) * W + (wp + kw), q] += g[kh] * g[kw]
    return L


def _build_P():
    Sh = np.zeros((H, H))
    for hp in range(H):
        for dh, wgt in ((-1, 1.0), (0, 2.0), (1, 1.0)):
            Sh[min(max(hp + dh, 0), H - 1), hp] += wgt
    Dw = np.zeros((W, W))
    for wp in range(W):
        for dw, wgt in ((-1, -1.0), (1, 1.0)):
            Dw[min(max(wp + dw, 0), W - 1), wp] += wgt
    return np.einsum("ha,wb->hwab", Sh, Dw).reshape(PIX, PIX)


def _m_band(d):
    return {0: 6.0, 1: 4.0, 2: 1.0}.get(abs(d), 0.0)


def _build_WM(first_tile=False, last_tile=False):
    Wm = np.zeros((128, 128))
    for i in range(128):
        for j in range(max(0, i - 2), min(128, i + 3)):
            Wm[i, j] = _m_band(i - j)
    if first_tile:
        Wm[0, 0] = 10.0
        Wm[0, 1] = Wm[1, 0] = 5.0
    if last_tile:
        Wm[-1, -1] = 10.0
        Wm[-1, -2] = Wm[-2, -1] = 5.0
    return Wm.astype(np.float32)


def _build_wxa(T):
    Mc = np.array([[1.0, 0.0], [4.0, 1.0]])
    blk = np.zeros((2 * T, 2 * T))
    for g in range(T):
        blk[2 * g:2 * g + 2, 2 * g:2 * g + 2] = Mc
    return blk.astype(np.float32)


def _round_f32r(v):
    i = np.ascontiguousarray(v.astype(np.float32)).view(np.int32)
    r = ((i.astype(np.int64) + 0x800) & ~0xFFF).astype(np.int32)
    out = r.view(np.float32).copy()
    out[np.asarray(v) == 0] = 0.0
    return out


def _build_lw():
    L = _build_L()
    hi = _round_f32r(L)
    lo = _round_f32r(L - hi.astype(np.float64))
    lw = np.zeros((CK, 2, NCHUNK, MOUT), dtype=np.float32)
    for c in range(NCHUNK):
        lw[:, 0, c, :] = hi[c * CK:(c + 1) * CK, :]
        lw[:, 1, c, :] = lo[c * CK:(c + 1) * CK, :]
    return lw


def _build_pw():
    P = _build_P().astype(np.float32)
    pw = np.zeros((CK, NCHUNK, 3, CK), dtype=np.float32)
    for c in range(NCHUNK):
        for mr in range(3):
            m = c + mr - 1
            if 0 <= m < NCHUNK:
                pw[:, c, mr, :] = P[c * CK:(c + 1) * CK, m * CK:(m + 1) * CK]
    return pw


# ------------------------------------------------------------ kernel builders

def build_pass1(b_loc):
    nc = bass.Bass("TRN2", target_bir_lowering=False, debug=False, num_devices=1)
    y_d = nc.dram_tensor("y", [b_loc, PIX], F32, kind="ExternalInput")
    mm_d = nc.dram_tensor("mm", [128, 2], F32, kind="ExternalOutput")
    a = b_loc // 128
    yv = y_d.ap().rearrange("(p a) f -> p (a f)", p=128)
    CH = 8 if a % 8 == 0 else 1
    n_ch = a // CH
    FD = CH * PIX
    from contextlib import ExitStack
    with tile.TileContext(nc) as tc:
        with ExitStack() as ctx:
            io = ctx.enter_context(tc.tile_pool(name="io", bufs=3))
            wk = ctx.enter_context(tc.tile_pool(name="wk", bufs=2))
            accp = ctx.enter_context(tc.tile_pool(name="accp", bufs=1))
            mx = accp.tile([128, n_ch], F32)
            mn = accp.tile([128, n_ch], F32)
            for i in range(n_ch):
                t = io.tile([128, FD], F32, tag="t")
                nc.sync.dma_start(t[:], yv[:, i * FD:(i + 1) * FD])
                neg = wk.tile([128, FD], F32, tag="neg")
                nc.scalar.mul(neg[:], t[:], -1.0)
                nc.vector.tensor_reduce(mx[:, i:i + 1], t[:], AX.X, ALU.max)
                nc.vector.tensor_reduce(mn[:, i:i + 1], neg[:], AX.X, ALU.max)
            out = accp.tile([128, 2], F32)
            nc.vector.tensor_reduce(out[:, 0:1], mx[:], AX.X, ALU.max)
            nc.vector.tensor_reduce(out[:, 1:2], mn[:], AX.X, ALU.max)
            nc.sync.dma_start(mm_d.ap(), out[:])
    return nc


def build_pass2(T):
    from contextlib import ExitStack
    nc = bass.Bass("TRN2", target_bir_lowering=False, debug=False, num_devices=1)
    x_d = nc.dram_tensor("x", [T * 128, PIX], F32, kind="ExternalInput")
    y_d = nc.dram_tensor("y", [T * 128, PIX], F32, kind="ExternalInput")
    xh_d = nc.dram_tensor("xh", [2, PIX], F32, kind="ExternalInput")
    yh_d = nc.dram_tensor("yh", [2, PIX], F32, kind="ExternalInput")
    cst_d = nc.dram_tensor("consts", [1, 8], F32, kind="ExternalInput")
    idn_d = nc.dram_tensor("idn", [128, 128], F32, kind="ExternalInput")
    lw_d = nc.dram_tensor("lw", [CK, 2, NCHUNK, MOUT], F32R, kind="ExternalInput")
    pw_d = nc.dram_tensor("pw", [CK, NCHUNK, 3, CK], F32R, kind="ExternalInput")
    wm_d = nc.dram_tensor("wm", [128, 128], F32, kind="ExternalInput")
    wmf_d = nc.dram_tensor("wmf", [128, 128], F32, kind="ExternalInput")
    wml_d = nc.dram_tensor("wml", [128, 128], F32, kind="ExternalInput")
    wxa_d = nc.dram_tensor("wxa", [2 * T, 2 * T], F32, kind="ExternalInput")

    o_mse = nc.dram_tensor("mse_h", [128, T], F32, kind="ExternalOutput")
    o_ssim = nc.dram_tensor("ssim_q", [128, T], F32, kind="ExternalOutput")
    o_gxy = nc.dram_tensor("gxy", [128, T], F32, kind="ExternalOutput")
    o_gxx = nc.dram_tensor("gxx", [128, T], F32, kind="ExternalOutput")
    o_gyy = nc.dram_tensor("gyy", [128, T], F32, kind="ExternalOutput")
    o_sx = nc.dram_tensor("sx", [128, T], F32, kind="ExternalOutput")
    o_sy = nc.dram_tensor("sy", [128, T], F32, kind="ExternalOutput")
    o_cross = nc.dram_tensor("cross", [128, 4], F32, kind="ExternalOutput")

    xv = x_d.ap().rearrange("(t p) f -> t p f", p=128)
    yv = y_d.ap().rearrange("(t p) f -> t p f", p=128)

    with tile.TileContext(nc) as tc:
        with ExitStack() as ctx:
            const = ctx.enter_context(tc.tile_pool(name="const", bufs=1))
            io = ctx.enter_context(tc.tile_pool(name="io", bufs=3))
            wk = ctx.enter_context(tc.tile_pool(name="wk", bufs=2))
            mp = ctx.enter_context(tc.tile_pool(name="mp", bufs=2))
            ps = ctx.enter_context(tc.tile_pool(name="ps", bufs=1, space="PSUM"))
            accp = ctx.enter_context(tc.tile_pool(name="accp", bufs=1))
            stp = ctx.enter_context(tc.tile_pool(name="stp", bufs=1))

            cst = const.tile([128, 8], F32)
            _cap = cst_d.ap()
            nc.sync.dma_start(cst[:], bass.AP(tensor=_cap.tensor, offset=_cap.offset,
                                              ap=[[0, 128], [1, 8]]))
            c1h, c2h, C1s, C2s = (cst[:, i:i + 1] for i in range(4))

            idn = const.tile([128, 128], F32)
            nc.sync.dma_start(idn[:], idn_d.ap())
            lw = const.tile([CK, 2, NCHUNK, MOUT], F32R)
            nc.sync.dma_start(lw[:], lw_d.ap())
            pw = const.tile([CK, NCHUNK, 3, CK], F32R)
            nc.sync.dma_start(pw[:], pw_d.ap())
            wm = const.tile([128, 128], F32)
            nc.sync.dma_start(wm[:], wm_d.ap())
            wmf = const.tile([128, 128], F32)
            nc.sync.dma_start(wmf[:], wmf_d.ap())
            wml = const.tile([128, 128], F32)
            nc.sync.dma_start(wml[:], wml_d.ap())
            wxa = const.tile([2 * T, 2 * T], F32)
            nc.sync.dma_start(wxa[:], wxa_d.ap())

            a_mse = accp.tile([128, T], F32)
            a_ssim = accp.tile([128, T], F32)
            a_gxy = accp.tile([128, T], F32)
            a_gxx = accp.tile([128, T], F32)
            a_gyy = accp.tile([128, T], F32)
            a_sx = accp.tile([128, T], F32)
            a_sy = accp.tile([128, T], F32)
            a_cross = accp.tile([128, 4], F32)
            for a in (a_mse, a_ssim, a_gxy, a_gxx, a_gyy, a_sx, a_sy, a_cross):
                nc.vector.memset(a[:], 0.0)

            st_fx = stp.tile([CK, NCHUNK, T, 2], F32R)
            st_fy = stp.tile([CK, NCHUNK, T, 2], F32R)
            st_lx = stp.tile([CK, NCHUNK, T, 2], F32R)
            st_ly = stp.tile([CK, NCHUNK, T, 2], F32R)
            nc.vector.memset(st_fx[:].bitcast(F32), 0.0)
            nc.vector.memset(st_fy[:].bitcast(F32), 0.0)

            def process_tile(t, xs, ys, nb):
                is_halo = t == T
                tp = ps.tile([CK, 2, NCHUNK, 128], F32, tag="pA")
                for c in range(NCHUNK):
                    nc.tensor.transpose(tp[:, 0, c, 0:nb], xs[0:nb, c * CK:(c + 1) * CK], idn[0:nb, 0:nb])
                    nc.tensor.transpose(tp[:, 1, c, 0:nb], ys[0:nb, c * CK:(c + 1) * CK], idn[0:nb, 0:nb])
                rhsP = wk.tile([CK, NCHUNK, 2, 128], F32R, tag="rhsP")
                nc.scalar.copy(rhsP[:, :, 0, 0:nb], tp[:, 0, :, 0:nb])
                nc.scalar.copy(rhsP[:, :, 1, 0:nb], tp[:, 1, :, 0:nb])
                xT = rhsP[:, :, 0, :]
                yT = rhsP[:, :, 1, :]

                if not is_halo:
                    cb = wk.tile([CK, NCHUNK, 4, 128], F32R, tag="cb")
                    sT = cb[:, :, 0, :]
                    dT = cb[:, :, 1, :]
                    nc.gpsimd.tensor_add(sT, xT, yT)
                    nc.gpsimd.tensor_sub(dT, xT, yT)
                    s2 = wk.tile([CK, NCHUNK, 128], F32, tag="s2")
                    d2 = wk.tile([CK, NCHUNK, 128], F32, tag="d2")
                    nc.scalar.activation(s2[:], sT.bitcast(F32), ACTF.Square, bias=0.0, scale=RS2)
                    nc.scalar.activation(d2[:], dT.bitcast(F32), ACTF.Square, bias=0.0, scale=RS2,
                                         accum_out=a_mse[:CK, t:t + 1])
                    nc.vector.tensor_add(cb[:, :, 2, :], s2[:], d2[:])
                    nc.vector.tensor_sub(cb[:, :, 3, :], s2[:], d2[:])

                    mmL = ps.tile([MK, MCH, 512], F32, tag="pB")
                    for m in range(MCH):
                        for c in range(NCHUNK):
                            for hl in range(2):
                                nc.tensor.matmul(
                                    mmL[:, m, :], lw[:, hl, c, m * MK:(m + 1) * MK],
                                    cb[:, c, :, :].rearrange("p a b -> p (a b)"),
                                    start=(c == 0 and hl == 0),
                                    stop=(c == NCHUNK - 1 and hl == 1))

                    sqS = mp.tile([MK, MCH, 128], F32, tag="sqS")
                    sqD = mp.tile([MK, MCH, 128], F32, tag="sqD")
                    eSc = mp.tile([MK, MCH, 128], F32, tag="eSc")
                    e2c = mp.tile([MK, MCH, 128], F32, tag="e2c")
                    nc.scalar.activation(sqS[:], mmL[:, :, 0:128], ACTF.Square, bias=0.0, scale=0.5)
                    nc.scalar.activation(sqD[:], mmL[:, :, 128:256], ACTF.Square, bias=0.0, scale=0.5)
                    nc.scalar.activation(eSc[:], mmL[:, :, 256:384], ACTF.Identity, bias=C2s[:MK], scale=1.0)
                    nc.scalar.activation(e2c[:], mmL[:, :, 384:512], ACTF.Identity, bias=c2h[:MK], scale=0.5)

                    n1 = mp.tile([MK, MCH, 128], F32, tag="n1")
                    n2 = mp.tile([MK, MCH, 128], F32, tag="n2")
                    q = mp.tile([MK, MCH, 128], F32, tag="q")
                    d1 = mp.tile([MK, MCH, 128], F32, tag="d1")
                    dd2 = mp.tile([MK, MCH, 128], F32, tag="dd2")
                    num = mp.tile([MK, MCH, 128], F32, tag="num")
                    den = mp.tile([MK, MCH, 128], F32, tag="den")
                    rcp = mp.tile([MK, MCH, 128], F32, tag="rcp")
                    scr = mp.tile([MK, MCH, 128], F32, tag="scr")
                    nc.vector.scalar_tensor_tensor(n1[:], sqS[:], c1h[:MK], sqD[:], ALU.add, ALU.subtract)
                    nc.vector.scalar_tensor_tensor(n2[:], sqD[:], 0.0, sqS[:], ALU.add, ALU.subtract)
                    nc.vector.tensor_add(n2[:], n2[:], e2c[:])
                    nc.gpsimd.tensor_add(q[:], sqS[:], sqD[:])
                    nc.vector.tensor_scalar(d1[:], q[:], 2.0, C1s[:MK], ALU.mult, ALU.add)
                    nc.vector.scalar_tensor_tensor(dd2[:], q[:], -2.0, eSc[:], ALU.mult, ALU.add)
                    nc.vector.tensor_mul(num[:], n1[:], n2[:])
                    nc.gpsimd.tensor_mul(den[:], d1[:], dd2[:])
                    nc.vector.reciprocal(rcp[:], den[:])
                    nc.vector.scalar_tensor_tensor(scr[:], num[:], 1.0, rcp[:], ALU.mult, ALU.mult,
                                                   accum_out=a_ssim[:MK, t:t + 1])

                dP = ps.tile([CK, NCHUNK, 256], F32, tag="pA")
                nwid = 256 if nb == 128 else 2 * nb
                for m in range(NCHUNK):
                    cs = [c for c in range(NCHUNK) if abs(c - m) <= 1]
                    for k, c in enumerate(cs):
                        nc.tensor.matmul(
                            dP[:, m, 0:nwid], pw[:, c, m - c + 1, :],
                            rhsP[:, c, :, 0:nb],
                            start=(k == 0), stop=(k == len(cs) - 1))

                if not is_halo:
                    rhsG = wk.tile([CK, NCHUNK, 258], F32R, tag="rhsG")
                    nc.scalar.copy(rhsG[:, :, 0:128], dP[:, :, 128:256])
                    nc.scalar.copy(rhsG[:, :, 128:256], dP[:, :, 0:128])
                    nc.vector.memset(rhsG[:, :, 256:257].bitcast(F32), 1.0)
                    nc.vector.memset(rhsG[:, :, 257:258].bitcast(F32), 0.0)
                    dyT = rhsG[:, :, 0:128]
                    dxT = rhsG[:, :, 128:256]
                    nc.vector.tensor_copy(st_lx[:, :, t, :], dxT[:, :, 126:128])
                    nc.vector.tensor_copy(st_ly[:, :, t, :], dyT[:, :, 126:128])
                    if t > 0:
                        nc.vector.tensor_copy(st_fx[:, :, t - 1, :], dxT[:, :, 0:2])
                        nc.vector.tensor_copy(st_fy[:, :, t - 1, :], dyT[:, :, 0:2])
                    gg = ps.tile([128, 2, 512], F32, tag="pB")
                    for c in range(NCHUNK):
                        nc.tensor.matmul(gg[:, 0, 0:258], dxT[:, c, :], rhsG[:, c, :],
                                         start=(c == 0), stop=(c == NCHUNK - 1))
                    for c in range(NCHUNK):
                        nc.tensor.matmul(gg[:, 1, 0:258], dyT[:, c, :], rhsG[:, c, :],
                                         start=(c == 0), stop=(c == NCHUNK - 1))
                    wsel = wmf if t == 0 else (wml if t == T - 1 else wm)
                    gs = mp.tile([128, 3, 128], F32, tag="gs")
                    nc.vector.scalar_tensor_tensor(gs[:, 0, :], gg[:, 0, 0:128], 1.0, wsel[:],
                                                   ALU.mult, ALU.mult, accum_out=a_gxy[:, t:t + 1])
                    nc.vector.scalar_tensor_tensor(gs[:, 1, :], gg[:, 0, 128:256], 1.0, wsel[:],
                                                   ALU.mult, ALU.mult, accum_out=a_gxx[:, t:t + 1])
                    nc.vector.scalar_tensor_tensor(gs[:, 2, :], gg[:, 1, 0:128], 1.0, wsel[:],
                                                   ALU.mult, ALU.mult, accum_out=a_gyy[:, t:t + 1])
                    nc.vector.tensor_copy(a_sx[:, t:t + 1], gg[:, 0, 256:257])
                    nc.vector.tensor_copy(a_sy[:, t:t + 1], gg[:, 1, 256:257])
                else:
                    hd = wk.tile([CK, NCHUNK, 4], F32R, tag="hd")
                    nc.scalar.copy(hd[:, :, 0:2], dP[:, :, 2:4])
                    nc.scalar.copy(hd[:, :, 2:4], dP[:, :, 0:2])
                    nc.vector.tensor_copy(st_fy[:, :, T - 1, :], hd[:, :, 0:2])
                    nc.vector.tensor_copy(st_fx[:, :, T - 1, :], hd[:, :, 2:4])

            for t in range(T):
                xs = io.tile([128, PIX], F32, tag="xs")
                ys = io.tile([128, PIX], F32, tag="ys")
                nc.sync.dma_start(xs[:], xv[t])
                nc.sync.dma_start(ys[:], yv[t])
                process_tile(t, xs, ys, 128)

            xs = io.tile([128, PIX], F32, tag="xs")
            ys = io.tile([128, PIX], F32, tag="ys")
            nc.vector.memset(xs[:], 0.0)
            nc.vector.memset(ys[:], 0.0)
            nc.sync.dma_start(xs[0:2, :], xh_d.ap())
            nc.sync.dma_start(ys[0:2, :], yh_d.ap())
            process_tile(T, xs, ys, 2)

            n2t = 2 * T
            sfx = st_fx[:].rearrange("p c t i -> p c (t i)")
            sfy = st_fy[:].rearrange("p c t i -> p c (t i)")
            slx = st_lx[:].rearrange("p c t i -> p c (t i)")
            sly = st_ly[:].rearrange("p c t i -> p c (t i)")
            rhsX = wk.tile([CK, NCHUNK, 2 * n2t], F32R, tag="rhsX")
            nc.vector.tensor_copy(rhsX[:, :, 0:n2t], sfy)
            nc.vector.tensor_copy(rhsX[:, :, n2t:2 * n2t], sfx)
            gX = ps.tile([n2t, 2, 2 * n2t], F32, tag="pB")
            for c in range(NCHUNK):
                nc.tensor.matmul(gX[:, 0, :], slx[:, c, :], rhsX[:, c, :],
                                 start=(c == 0), stop=(c == NCHUNK - 1))
            for c in range(NCHUNK):
                nc.tensor.matmul(gX[:, 1, :], sly[:, c, :], rhsX[:, c, :],
                                 start=(c == 0), stop=(c == NCHUNK - 1))
            xscr = mp.tile([n2t, 4, n2t], F32, tag="xscr")
            nc.vector.scalar_tensor_tensor(xscr[:, 0, :], gX[:, 0, 0:n2t], 1.0, wxa[:],
                                           ALU.mult, ALU.mult, accum_out=a_cross[0:n2t, 0:1])
            nc.vector.scalar_tensor_tensor(xscr[:, 1, :], gX[:, 0, n2t:2 * n2t], 2.0, wxa[:],
                                           ALU.mult, ALU.mult, accum_out=a_cross[0:n2t, 1:2])
            nc.vector.scalar_tensor_tensor(xscr[:, 2, :], gX[:, 1, 0:n2t], 2.0, wxa[:],
                                           ALU.mult, ALU.mult, accum_out=a_cross[0:n2t, 2:3])
            nc.vector.scalar_tensor_tensor(xscr[:, 3, :], gX[:, 1, n2t:2 * n2t], 1.0, wxa[:],
                                           ALU.mult, ALU.mult, accum_out=a_cross[0:n2t, 3:4])

            nc.sync.dma_start(o_mse.ap(), a_mse[:])
            nc.sync.dma_start(o_ssim.ap(), a_ssim[:])
            nc.sync.dma_start(o_gxy.ap(), a_gxy[:])
            nc.sync.dma_start(o_gxx.ap(), a_gxx[:])
            nc.sync.dma_start(o_gyy.ap(), a_gyy[:])
            nc.sync.dma_start(o_sx.ap(), a_sx[:])
            nc.sync.dma_start(o_sy.ap(), a_sy[:])
            nc.sync.dma_start(o_cross.ap(), a_cross[:])
    return nc


# ---------------------------------------------------------------- driver

_CACHE = {}


def _get_kernels():
    if "k" not in _CACHE:
        _CACHE["k"] = (build_pass1(B_LOC), build_pass2(T_TILES))
    return _CACHE["k"]


def kernel(output, target):
    output = np.ascontiguousarray(np.asarray(output, dtype=np.float32))
    target = np.ascontiguousarray(np.asarray(target, dtype=np.float32))
    assert output.shape == (B_GLOB, PIX) and target.shape == (B_GLOB, PIX)

    nc1, nc2 = _get_kernels()

    xs = output.reshape(N_CORES, B_LOC, PIX)
    ys = target.reshape(N_CORES, B_LOC, PIX)
    core_ids = list(range(N_CORES))

    # ---- pass 1: minmax(target)
    r1 = run_bass_kernel_spmd(nc1, [{"y": ys[k]} for k in range(N_CORES)],
                              core_ids=core_ids).results
    ymax = max(float(r["mm"][:, 0].max()) for r in r1)
    ymin = -max(float(r["mm"][:, 1].max()) for r in r1)
    dr = ymax - ymin
    C1 = (K1 * dr) ** 2
    C2 = (K2 * dr) ** 2

    # ---- pass 2
    wm_int = _build_WM()
    consts_common = {
        "consts": np.array([[C1 / 2, C2 / 2, C1, C2, 0, 0, 0, 0]], dtype=np.float32),
        "idn": np.eye(128, dtype=np.float32),
        "lw": _build_lw(),
        "pw": _build_pw(),
        "wm": wm_int,
        "wxa": _build_wxa(T_TILES),
    }
    zh = np.zeros((2, PIX), dtype=np.float32)
    in_maps = []
    for k in range(N_CORES):
        e = (k + 1) * B_LOC
        halo_x = output[e:e + 2] if k < N_CORES - 1 else zh
        halo_y = target[e:e + 2] if k < N_CORES - 1 else zh
        in_maps.append({
            "x": xs[k], "y": ys[k],
            "xh": np.ascontiguousarray(halo_x), "yh": np.ascontiguousarray(halo_y),
            "wmf": _build_WM(first_tile=True) if k == 0 else wm_int,
            "wml": _build_WM(last_tile=True) if k == N_CORES - 1 else wm_int,
            **consts_common,
        })
    r2 = run_bass_kernel_spmd(nc2, in_maps, core_ids=core_ids).results

    # ---- host combine (float64)
    tot = dict(mse_sum=0.0, ssim_sum=0.0, sxy=0.0, sxx=0.0, syy=0.0, sx=0.0, sy=0.0)
    for r in r2:
        cr = r["cross"].astype(np.float64)
        tot["mse_sum"] += 2.0 * r["mse_h"].astype(np.float64).sum()
        tot["ssim_sum"] += 4.0 * r["ssim_q"].astype(np.float64).sum()
        tot["sxy"] += r["gxy"].astype(np.float64).sum() + cr[:, 0].sum() + cr[:, 3].sum()
        tot["sxx"] += r["gxx"].astype(np.float64).sum() + cr[:, 1].sum()
        tot["syy"] += r["gyy"].astype(np.float64).sum() + cr[:, 2].sum()
        tot["sx"] += 4.0 * r["sx"].astype(np.float64).sum()
        tot["sy"] += 4.0 * r["sy"].astype(np.float64).sum()

    n = float(B_GLOB * PIX)
    mse = tot["mse_sum"] / n
    psnr = -10.0 * np.log10(mse)
    ssim_val = tot["ssim_sum"] / (B_GLOB * 324.0)
    cov = tot["sxy"] - tot["sx"] * tot["sy"] / n
    vx = tot["sxx"] - tot["sx"] ** 2 / n
    vy = tot["syy"] - tot["sy"] ** 2 / n
    epi = cov / np.sqrt(vx * vy)
    loss = MSE_W * mse + SSIM_W * (1.0 - ssim_val) + EPI_W * epi + PSNR_W * psnr
    return np.float32(loss)


# revision 2
# speedup vs baseline: 1.2245x; 1.2245x over previous
"""Trainium2 Bass kernel for CombinedMSESSIMLoss (MSE + SSIM + EPI + PSNR).

Contract: kernel(output, target) -> np.float32 scalar loss, computed on 8
NeuronCores, data-parallel over the batch dim (65536 images of 28x28).

Structure:
  pass 1 (per core): max(target), max(-target) over the 8192-image shard.
  host: combine -> data_range -> C1, C2 ssim constants.
  pass 2 (per core): all remaining partial sums in one streamed kernel:
    - transposes each 128-image tile to pixel-major via the PE,
    - SSIM 11x11 valid gaussian filtering as dense [112,108] PE matmuls over
      {x+y, x-y, x^2+y^2, 2xy} (error-compensated f32r weights),
    - ssim rational map on DVE/ACT/GPSIMD with per-tile accumulators,
    - per-image sobel d-maps as banded PE matmuls (exact integer weights),
    - batch-axis [1,2,1] smoothing handled algebraically: sum S_x S_y =
      sum_{|b-b'|<=2} M[b,b'] dx[b].dy[b'] with M = A A^T pentadiagonal,
      evaluated via PE gram matrices + weighted reductions; tile/core
      boundary pairs via stashed edge columns + one cross-gram at the end.
  host: assemble loss in float64 (log10, sqrt, divisions).
"""
import json

import numpy as np

import concourse.bass as bass
import concourse.tile as tile
from concourse import mybir

F32 = mybir.dt.float32
F32R = mybir.dt.float32r
ALU = mybir.AluOpType
ACTF = mybir.ActivationFunctionType
AX = mybir.AxisListType

H = W = 28
PIX = H * W
NCHUNK = 7
CK = 112
MOUT = 324
MCH = 3
MK = 108
WIN, SIGMA, K1, K2 = 11, 1.5, 0.01, 0.03
OW = 18
RS2 = float(1.0 / np.sqrt(2.0))

B_GLOB = 65536
N_CORES = 8
B_LOC = B_GLOB // N_CORES     # 8192
T_TILES = B_LOC // 128        # 64

MSE_W, SSIM_W, EPI_W, PSNR_W = 1.0, 0.5, 0.1, 0.01


# ---------------------------------------------------------------- walrus fix
# This walrus build rejects >1 sync-wait per instruction; split extra waits
# onto single-wait NoOps ahead of the instruction.
_orig_to_json_bytes = bass.Bass.to_json_bytes


def _split_waits(obj):
    if isinstance(obj, dict):
        ilist = obj.get("instructions")
        if isinstance(ilist, list):
            newlist = []
            for ins in ilist:
                try:
                    w = ins.get("sync_info", {}).get("on_wait", [])
                except AttributeError:
                    w = []
                if isinstance(w, list) and len(w) > 1:
                    for k, wt in enumerate(w[:-1]):
                        newlist.append({
                            "debug": ins.get("debug", 0),
                            "engine": ins["engine"],
                            "ins": [], "outs": [],
                            "name": str(ins["name"]) + f"_wsplit{k}",
                            "opcode": "NoOp",
                            "sync_info": {"on_update": [], "on_wait": [wt]},
                        })
                    ins["sync_info"]["on_wait"] = [w[-1]]
                newlist.append(ins)
            obj["instructions"] = newlist
        for v in obj.values():
            _split_waits(v)
    elif isinstance(obj, list):
        for v in obj:
            _split_waits(v)


def _patched_to_json_bytes(self, *a, **k):
    data = json.loads(_orig_to_json_bytes(self, *a, **k))
    _split_waits(data)
    return json.dumps(data).encode()


bass.Bass.to_json_bytes = _patched_to_json_bytes


# ----------------------------------------------------------- const builders

def _gauss1d():
    c = np.arange(WIN, dtype=np.float64) - WIN // 2
    g = np.exp(-(c ** 2) / (2.0 * SIGMA ** 2))
    return g / g.sum()


def _build_L():
    g = _gauss1d()
    L = np.zeros((PIX, MOUT), dtype=np.float64)
    for hp in range(OW):
        for wp in range(OW):
            q = hp * OW + wp
            for kh in range(WIN):
                for kw in range(WIN):
                    L[(hp + kh) * W + (wp + kw), q] += g[kh] * g[kw]
    return L


def _build_P():
    Sh = np.zeros((H, H))
    for hp in range(H):
        for dh, wgt in ((-1, 1.0), (0, 2.0), (1, 1.0)):
            Sh[min(max(hp + dh, 0), H - 1), hp] += wgt
    Dw = np.zeros((W, W))
    for wp in range(W):
        for dw, wgt in ((-1, -1.0), (1, 1.0)):
            Dw[min(max(wp + dw, 0), W - 1), wp] += wgt
    return np.einsum("ha,wb->hwab", Sh, Dw).reshape(PIX, PIX)


def _m_band(d):
    return {0: 6.0, 1: 4.0, 2: 1.0}.get(abs(d), 0.0)


def _build_WM(first_tile=False, last_tile=False):
    Wm = np.zeros((128, 128))
    for i in range(128):
        for j in range(max(0, i - 2), min(128, i + 3)):
            Wm[i, j] = _m_band(i - j)
    if first_tile:
        Wm[0, 0] = 10.0
        Wm[0, 1] = Wm[1, 0] = 5.0
    if last_tile:
        Wm[-1, -1] = 10.0
        Wm[-1, -2] = Wm[-2, -1] = 5.0
    return Wm.astype(np.float32)


def _build_wxa(T):
    Mc = np.array([[1.0, 0.0], [4.0, 1.0]])
    blk = np.zeros((2 * T, 2 * T))
    for g in range(T):
        blk[2 * g:2 * g + 2, 2 * g:2 * g + 2] = Mc
    return blk.astype(np.float32)


def _round_f32r(v):
    i = np.ascontiguousarray(v.astype(np.float32)).view(np.int32)
    r = ((i.astype(np.int64) + 0x800) & ~0xFFF).astype(np.int32)
    out = r.view(np.float32).copy()
    out[np.asarray(v) == 0] = 0.0
    return out


def _build_lw():
    L = _build_L()
    hi = _round_f32r(L)
    lo = _round_f32r(L - hi.astype(np.float64))
    lw = np.zeros((CK, 2, NCHUNK, MOUT), dtype=np.float32)
    for c in range(NCHUNK):
        lw[:, 0, c, :] = hi[c * CK:(c + 1) * CK, :]
        lw[:, 1, c, :] = lo[c * CK:(c + 1) * CK, :]
    return lw


def _build_pw():
    P = _build_P().astype(np.float32)
    pw = np.zeros((CK, NCHUNK, 3, CK), dtype=np.float32)
    for c in range(NCHUNK):
        for mr in range(3):
            m = c + mr - 1
            if 0 <= m < NCHUNK:
                pw[:, c, mr, :] = P[c * CK:(c + 1) * CK, m * CK:(m + 1) * CK]
    return pw


# ------------------------------------------------------------ kernel builders

def build_pass1(b_loc):
    nc = bass.Bass("TRN2", target_bir_lowering=False, debug=False, num_devices=1)
    y_d = nc.dram_tensor("y", [b_loc, PIX], F32, kind="ExternalInput")
    mm_d = nc.dram_tensor("mm", [128, 2], F32, kind="ExternalOutput")
    a = b_loc // 128
    yv = y_d.ap().rearrange("(p a) f -> p (a f)", p=128)
    CH = 8 if a % 8 == 0 else 1
    n_ch = a // CH
    FD = CH * PIX
    from contextlib import ExitStack
    with tile.TileContext(nc) as tc:
        with ExitStack() as ctx:
            io = ctx.enter_context(tc.tile_pool(name="io", bufs=3))
            wk = ctx.enter_context(tc.tile_pool(name="wk", bufs=2))
            accp = ctx.enter_context(tc.tile_pool(name="accp", bufs=1))
            mx = accp.tile([128, n_ch], F32)
            mn = accp.tile([128, n_ch], F32)
            for i in range(n_ch):
                t = io.tile([128, FD], F32, tag="t")
                nc.sync.dma_start(t[:], yv[:, i * FD:(i + 1) * FD])
                neg = wk.tile([128, FD], F32, tag="neg")
                nc.scalar.mul(neg[:], t[:], -1.0)
                nc.vector.tensor_reduce(mx[:, i:i + 1], t[:], AX.X, ALU.max)
                nc.vector.tensor_reduce(mn[:, i:i + 1], neg[:], AX.X, ALU.max)
            out = accp.tile([128, 2], F32)
            nc.vector.tensor_reduce(out[:, 0:1], mx[:], AX.X, ALU.max)
            nc.vector.tensor_reduce(out[:, 1:2], mn[:], AX.X, ALU.max)
            nc.sync.dma_start(mm_d.ap(), out[:])
    return nc


def build_pass2(T):
    from contextlib import ExitStack
    nc = bass.Bass("TRN2", target_bir_lowering=False, debug=False, num_devices=1)
    x_d = nc.dram_tensor("x", [T * 128, PIX], F32, kind="ExternalInput")
    y_d = nc.dram_tensor("y", [T * 128, PIX], F32, kind="ExternalInput")
    xh_d = nc.dram_tensor("xh", [2, PIX], F32, kind="ExternalInput")
    yh_d = nc.dram_tensor("yh", [2, PIX], F32, kind="ExternalInput")
    cst_d = nc.dram_tensor("consts", [1, 8], F32, kind="ExternalInput")
    idn_d = nc.dram_tensor("idn", [128, 128], F32, kind="ExternalInput")
    lw_d = nc.dram_tensor("lw", [CK, 2, NCHUNK, MOUT], F32R, kind="ExternalInput")
    pw_d = nc.dram_tensor("pw", [CK, NCHUNK, 3, CK], F32R, kind="ExternalInput")
    wm_d = nc.dram_tensor("wm", [128, 128], F32, kind="ExternalInput")
    wmf_d = nc.dram_tensor("wmf", [128, 128], F32, kind="ExternalInput")
    wml_d = nc.dram_tensor("wml", [128, 128], F32, kind="ExternalInput")
    wxa_d = nc.dram_tensor("wxa", [2 * T, 2 * T], F32, kind="ExternalInput")

    o_mse = nc.dram_tensor("mse_h", [128, T], F32, kind="ExternalOutput")
    o_ssim = nc.dram_tensor("ssim_q", [128, T], F32, kind="ExternalOutput")
    o_gxy = nc.dram_tensor("gxy", [128, T], F32, kind="ExternalOutput")
    o_gxx = nc.dram_tensor("gxx", [128, T], F32, kind="ExternalOutput")
    o_gyy = nc.dram_tensor("gyy", [128, T], F32, kind="ExternalOutput")
    o_sx = nc.dram_tensor("sx", [128, T], F32, kind="ExternalOutput")
    o_sy = nc.dram_tensor("sy", [128, T], F32, kind="ExternalOutput")
    o_cross = nc.dram_tensor("cross", [128, 4], F32, kind="ExternalOutput")

    xv = x_d.ap().rearrange("(t p) f -> t p f", p=128)
    yv = y_d.ap().rearrange("(t p) f -> t p f", p=128)

    with tile.TileContext(nc) as tc:
        with ExitStack() as ctx:
            const = ctx.enter_context(tc.tile_pool(name="const", bufs=1))
            io = ctx.enter_context(tc.tile_pool(name="io", bufs=3))
            wk = ctx.enter_context(tc.tile_pool(name="wk", bufs=2))
            mp = ctx.enter_context(tc.tile_pool(name="mp", bufs=2))
            ps = ctx.enter_context(tc.tile_pool(name="ps", bufs=1, space="PSUM"))
            accp = ctx.enter_context(tc.tile_pool(name="accp", bufs=1))
            stp = ctx.enter_context(tc.tile_pool(name="stp", bufs=1))

            cst = const.tile([128, 8], F32)
            _cap = cst_d.ap()
            nc.sync.dma_start(cst[:], bass.AP(tensor=_cap.tensor, offset=_cap.offset,
                                              ap=[[0, 128], [1, 8]]))
            c1h, c2h, C1s, C2s = (cst[:, i:i + 1] for i in range(4))

            idn = const.tile([128, 128], F32)
            nc.sync.dma_start(idn[:], idn_d.ap())
            lw = const.tile([CK, 2, NCHUNK, MOUT], F32R)
            nc.sync.dma_start(lw[:], lw_d.ap())
            pw = const.tile([CK, NCHUNK, 3, CK], F32R)
            nc.sync.dma_start(pw[:], pw_d.ap())
            wm = const.tile([128, 128], F32)
            nc.sync.dma_start(wm[:], wm_d.ap())
            wmf = const.tile([128, 128], F32)
            nc.sync.dma_start(wmf[:], wmf_d.ap())
            wml = const.tile([128, 128], F32)
            nc.sync.dma_start(wml[:], wml_d.ap())
            wxa = const.tile([2 * T, 2 * T], F32)
            nc.sync.dma_start(wxa[:], wxa_d.ap())

            a_mse = accp.tile([128, T], F32)
            a_ssim = accp.tile([128, T], F32)
            a_gxy = accp.tile([128, T], F32)
            a_gxx = accp.tile([128, T], F32)
            a_gyy = accp.tile([128, T], F32)
            a_sx = accp.tile([128, T], F32)
            a_sy = accp.tile([128, T], F32)
            a_cross = accp.tile([128, 4], F32)
            for a in (a_mse, a_ssim, a_gxy, a_gxx, a_gyy, a_sx, a_sy, a_cross):
                nc.vector.memset(a[:], 0.0)

            st_fx = stp.tile([CK, NCHUNK, T, 2], F32R)
            st_fy = stp.tile([CK, NCHUNK, T, 2], F32R)
            st_lx = stp.tile([CK, NCHUNK, T, 2], F32R)
            st_ly = stp.tile([CK, NCHUNK, T, 2], F32R)
            nc.vector.memset(st_fx[:].bitcast(F32), 0.0)
            nc.vector.memset(st_fy[:].bitcast(F32), 0.0)

            def process_tile(t, xs, ys, nb):
                is_halo = t == T
                tp = ps.tile([CK, 2, NCHUNK, 128], F32, tag="pA")
                for c in range(NCHUNK):
                    nc.tensor.transpose(tp[:, 0, c, 0:nb], xs[0:nb, c * CK:(c + 1) * CK], idn[0:nb, 0:nb])
                    nc.tensor.transpose(tp[:, 1, c, 0:nb], ys[0:nb, c * CK:(c + 1) * CK], idn[0:nb, 0:nb])
                rhsP = wk.tile([CK, NCHUNK, 2, 128], F32R, tag="rhsP")
                nc.scalar.copy(rhsP[:, :, 0, 0:nb], tp[:, 0, :, 0:nb])
                nc.scalar.copy(rhsP[:, :, 1, 0:nb], tp[:, 1, :, 0:nb])
                xT = rhsP[:, :, 0, :]
                yT = rhsP[:, :, 1, :]

                if not is_halo:
                    cb = wk.tile([CK, NCHUNK, 4, 128], F32R, tag="cb")
                    sT = cb[:, :, 0, :]
                    dT = cb[:, :, 1, :]
                    nc.gpsimd.tensor_add(sT, xT, yT)
                    nc.gpsimd.tensor_sub(dT, xT, yT)
                    s2 = wk.tile([CK, NCHUNK, 128], F32, tag="s2")
                    d2 = wk.tile([CK, NCHUNK, 128], F32, tag="d2")
                    nc.scalar.activation(s2[:], sT.bitcast(F32), ACTF.Square, bias=0.0, scale=RS2)
                    nc.scalar.activation(d2[:], dT.bitcast(F32), ACTF.Square, bias=0.0, scale=RS2,
                                         accum_out=a_mse[:CK, t:t + 1])
                    nc.vector.tensor_add(cb[:, :, 2, :], s2[:], d2[:])
                    nc.vector.tensor_sub(cb[:, :, 3, :], s2[:], d2[:])

                    mmL = ps.tile([MK, MCH, 512], F32, tag="pB")
                    for m in range(MCH):
                        for c in range(NCHUNK):
                            for hl in range(2):
                                nc.tensor.matmul(
                                    mmL[:, m, :], lw[:, hl, c, m * MK:(m + 1) * MK],
                                    cb[:, c, :, :].rearrange("p a b -> p (a b)"),
                                    start=(c == 0 and hl == 0),
                                    stop=(c == NCHUNK - 1 and hl == 1))

                    sqS = mp.tile([MK, MCH, 128], F32, tag="sqS")
                    sqD = mp.tile([MK, MCH, 128], F32, tag="sqD")
                    eSc = mp.tile([MK, MCH, 128], F32, tag="eSc")
                    e2c = mp.tile([MK, MCH, 128], F32, tag="e2c")
                    nc.scalar.activation(sqS[:], mmL[:, :, 0:128], ACTF.Square, bias=0.0, scale=0.5)
                    nc.scalar.activation(sqD[:], mmL[:, :, 128:256], ACTF.Square, bias=0.0, scale=0.5)
                    nc.scalar.activation(eSc[:], mmL[:, :, 256:384], ACTF.Identity, bias=C2s[:MK], scale=1.0)
                    nc.scalar.activation(e2c[:], mmL[:, :, 384:512], ACTF.Identity, bias=c2h[:MK], scale=0.5)

                    n1 = mp.tile([MK, MCH, 128], F32, tag="n1")
                    n2 = mp.tile([MK, MCH, 128], F32, tag="n2")
                    q = mp.tile([MK, MCH, 128], F32, tag="q")
                    d1 = mp.tile([MK, MCH, 128], F32, tag="d1")
                    dd2 = mp.tile([MK, MCH, 128], F32, tag="dd2")
                    num = mp.tile([MK, MCH, 128], F32, tag="num")
                    den = mp.tile([MK, MCH, 128], F32, tag="den")
                    rcp = mp.tile([MK, MCH, 128], F32, tag="rcp")
                    scr = mp.tile([MK, MCH, 128], F32, tag="scr")
                    nc.vector.scalar_tensor_tensor(n1[:], sqS[:], c1h[:MK], sqD[:], ALU.add, ALU.subtract)
                    nc.vector.scalar_tensor_tensor(n2[:], sqD[:], 0.0, sqS[:], ALU.add, ALU.subtract)
                    nc.vector.tensor_add(n2[:], n2[:], e2c[:])
                    nc.gpsimd.tensor_add(q[:], sqS[:], sqD[:])
                    nc.vector.tensor_scalar(d1[:], q[:], 2.0, C1s[:MK], ALU.mult, ALU.add)
                    nc.vector.scalar_tensor_tensor(dd2[:], q[:], -2.0, eSc[:], ALU.mult, ALU.add)
                    nc.vector.tensor_mul(num[:], n1[:], n2[:])
                    nc.gpsimd.tensor_mul(den[:], d1[:], dd2[:])
                    nc.vector.reciprocal(rcp[:], den[:])
                    nc.vector.scalar_tensor_tensor(scr[:], num[:], 1.0, rcp[:], ALU.mult, ALU.mult,
                                                   accum_out=a_ssim[:MK, t:t + 1])

                dP = ps.tile([CK, NCHUNK, 256], F32, tag="pA")
                nwid = 256 if nb == 128 else 2 * nb
                for m in range(NCHUNK):
                    cs = [c for c in range(NCHUNK) if abs(c - m) <= 1]
                    for k, c in enumerate(cs):
                        nc.tensor.matmul(
                            dP[:, m, 0:nwid], pw[:, c, m - c + 1, :],
                            rhsP[:, c, :, 0:nb],
                            start=(k == 0), stop=(k == len(cs) - 1))

                if not is_halo:
                    rhsG = wk.tile([CK, NCHUNK, 258], F32R, tag="rhsG")
                    nc.scalar.copy(rhsG[:, :, 0:128], dP[:, :, 128:256])
                    nc.scalar.copy(rhsG[:, :, 128:256], dP[:, :, 0:128])
                    nc.vector.memset(rhsG[:, :, 256:257].bitcast(F32), 1.0)
                    nc.vector.memset(rhsG[:, :, 257:258].bitcast(F32), 0.0)
                    dyT = rhsG[:, :, 0:128]
                    dxT = rhsG[:, :, 128:256]
                    nc.vector.tensor_copy(st_lx[:, :, t, :], dxT[:, :, 126:128])
                    nc.vector.tensor_copy(st_ly[:, :, t, :], dyT[:, :, 126:128])
                    if t > 0:
                        nc.vector.tensor_copy(st_fx[:, :, t - 1, :], dxT[:, :, 0:2])
                        nc.vector.tensor_copy(st_fy[:, :, t - 1, :], dyT[:, :, 0:2])
                    gg = ps.tile([128, 2, 512], F32, tag="pB")
                    for c in range(NCHUNK):
                        nc.tensor.matmul(gg[:, 0, 0:258], dxT[:, c, :], rhsG[:, c, :],
                                         start=(c == 0), stop=(c == NCHUNK - 1))
                    for c in range(NCHUNK):
                        nc.tensor.matmul(gg[:, 1, 0:258], dyT[:, c, :], rhsG[:, c, :],
                                         start=(c == 0), stop=(c == NCHUNK - 1))
                    wsel = wmf if t == 0 else (wml if t == T - 1 else wm)
                    gs = mp.tile([128, 3, 128], F32, tag="gs")
                    nc.vector.scalar_tensor_tensor(gs[:, 0, :], gg[:, 0, 0:128], 1.0, wsel[:],
                                                   ALU.mult, ALU.mult, accum_out=a_gxy[:, t:t + 1])
                    nc.vector.scalar_tensor_tensor(gs[:, 1, :], gg[:, 0, 128:256], 1.0, wsel[:],
                                                   ALU.mult, ALU.mult, accum_out=a_gxx[:, t:t + 1])
                    nc.vector.scalar_tensor_tensor(gs[:, 2, :], gg[:, 1, 0:128], 1.0, wsel[:],
                                                   ALU.mult, ALU.mult, accum_out=a_gyy[:, t:t + 1])
                    nc.vector.tensor_copy(a_sx[:, t:t + 1], gg[:, 0, 256:257])
                    nc.vector.tensor_copy(a_sy[:, t:t + 1], gg[:, 1, 256:257])
                else:
                    hd = wk.tile([CK, NCHUNK, 4], F32R, tag="hd")
                    nc.scalar.copy(hd[:, :, 0:2], dP[:, :, 2:4])
                    nc.scalar.copy(hd[:, :, 2:4], dP[:, :, 0:2])
                    nc.vector.tensor_copy(st_fy[:, :, T - 1, :], hd[:, :, 0:2])
                    nc.vector.tensor_copy(st_fx[:, :, T - 1, :], hd[:, :, 2:4])

            for t in range(T):
                xs = io.tile([128, PIX], F32, tag="xs")
                ys = io.tile([128, PIX], F32, tag="ys")
                nc.sync.dma_start(xs[:], xv[t])
                nc.sync.dma_start(ys[:], yv[t])
                process_tile(t, xs, ys, 128)

            xs = io.tile([128, PIX], F32, tag="xs")
            ys = io.tile([128, PIX], F32, tag="ys")
            nc.vector.memset(xs[:], 0.0)
            nc.vector.memset(ys[:], 0.0)
            nc.sync.dma_start(xs[0:2, :], xh_d.ap())
            nc.sync.dma_start(ys[0:2, :], yh_d.ap())
            process_tile(T, xs, ys, 2)

            n2t = 2 * T
            sfx = st_fx[:].rearrange("p c t i -> p c (t i)")
            sfy = st_fy[:].rearrange("p c t i -> p c (t i)")
            slx = st_lx[:].rearrange("p c t i -> p c (t i)")
            sly = st_ly[:].rearrange("p c t i -> p c (t i)")
            rhsX = wk.tile([CK, NCHUNK, 2 * n2t], F32R, tag="rhsX")
            nc.vector.tensor_copy(rhsX[:, :, 0:n2t], sfy)
            nc.vector.tensor_copy(rhsX[:, :, n2t:2 * n2t], sfx)
            gX = ps.tile([n2t, 2, 2 * n2t], F32, tag="pB")
            for c in range(NCHUNK):
                nc.tensor.matmul(gX[:, 0, :], slx[:, c, :], rhsX[:, c, :],
                                 start=(c == 0), stop=(c == NCHUNK - 1))
            for c in range(NCHUNK):
                nc.tensor.matmul(gX[:, 1, :], sly[:, c, :], rhsX[:, c, :],
                                 start=(c == 0), stop=(c == NCHUNK - 1))
            xscr = mp.tile([n2t, 4, n2t], F32, tag="xscr")
            nc.vector.scalar_tensor_tensor(xscr[:, 0, :], gX[:, 0, 0:n2t], 1.0, wxa[:],
                                           ALU.mult, ALU.mult, accum_out=a_cross[0:n2t, 0:1])
            nc.vector.scalar_tensor_tensor(xscr[:, 1, :], gX[:, 0, n2t:2 * n2t], 2.0, wxa[:],
                                           ALU.mult, ALU.mult, accum_out=a_cross[0:n2t, 1:2])
            nc.vector.scalar_tensor_tensor(xscr[:, 2, :], gX[:, 1, 0:n2t], 2.0, wxa[:],
                                           ALU.mult, ALU.mult, accum_out=a_cross[0:n2t, 2:3])
            nc.vector.scalar_tensor_tensor(xscr[:, 3, :], gX[:, 1, n2t:2 * n2t], 1.0, wxa[:],
                                           ALU.mult, ALU.mult, accum_out=a_cross[0:n2t, 3:4])

            nc.sync.dma_start(o_mse.ap(), a_mse[:])
            nc.sync.dma_start(o_ssim.ap(), a_ssim[:])
            nc.sync.dma_start(o_gxy.ap(), a_gxy[:])
            nc.sync.dma_start(o_gxx.ap(), a_gxx[:])
            nc.sync.dma_start(o_gyy.ap(), a_gyy[:])
            nc.sync.dma_start(o_sx.ap(), a_sx[:])
            nc.sync.dma_start(o_sy.ap(), a_sy[:])
            nc.sync.dma_start(o_cross.ap(), a_cross[:])
    return nc


# ---------------------------------------------------------------- driver


class _Runner:
    """Caches the shard_map-jitted executable for a built Bass module."""

    def __init__(self, nc):
        import jax
        from jax.sharding import Mesh, PartitionSpec
        from jax.experimental.shard_map import shard_map
        from concourse.bass2jax import (_bass_exec_p, install_neuronx_cc_hook,
                                        partition_id_tensor)
        install_neuronx_cc_hook()
        self.jax = jax
        partition_name = (nc.partition_id_tensor.name
                          if nc.partition_id_tensor else None)
        in_names, out_names, out_avals, zero_outs = [], [], [], []
        for alloc in nc.m.functions[0].allocations:
            if not isinstance(alloc, mybir.MemoryLocationSet):
                continue
            name = alloc.memorylocations[0].name
            if alloc.kind == "ExternalInput":
                if name != partition_name:
                    in_names.append(name)
            elif alloc.kind == "ExternalOutput":
                out_names.append(name)
                shape = tuple(alloc.tensor_shape)
                dtype = mybir.dt.np(alloc.dtype)
                out_avals.append(jax.core.ShapedArray(shape, dtype))
                zero_outs.append(np.zeros(shape, dtype))
        self.in_names = in_names
        self.out_names = out_names
        self.out_avals = out_avals
        n_params = len(in_names)
        n_outs = len(out_avals)
        all_in = list(in_names) + list(out_names)
        if partition_name is not None:
            all_in.append(partition_name)

        def _body(*args):
            operands = list(args)
            if partition_name is not None:
                operands.append(partition_id_tensor())
            return tuple(_bass_exec_p.bind(
                *operands, out_avals=tuple(out_avals), in_names=tuple(all_in),
                out_names=tuple(out_names), lowering_input_output_aliases=(),
                sim_require_finite=True, sim_require_nnan=True, nc=nc))

        devices = jax.devices()[:N_CORES]
        self.mesh = Mesh(np.asarray(devices), ("core",))
        self.sharding = jax.sharding.NamedSharding(self.mesh, PartitionSpec("core"))
        in_specs = (PartitionSpec("core"),) * (n_params + n_outs)
        out_specs = (PartitionSpec("core"),) * n_outs
        self.fn = jax.jit(
            shard_map(_body, mesh=self.mesh, in_specs=in_specs,
                      out_specs=out_specs, check_rep=False),
            keep_unused=True)
        self.zero_dev = [
            jax.device_put(np.zeros((N_CORES * z.shape[0],) + z.shape[1:], z.dtype),
                           self.sharding) for z in zero_outs]

    def put(self, arr):
        return self.jax.device_put(arr, self.sharding)

    def run(self, concat_inputs):
        args = [concat_inputs[n] if not isinstance(concat_inputs[n], np.ndarray)
                else self.put(concat_inputs[n]) for n in self.in_names]
        outs = self.fn(*args, *self.zero_dev)
        outs = [np.asarray(o) for o in outs]
        return [
            {n: outs[i].reshape((N_CORES, outs[i].shape[0] // N_CORES)
                                + outs[i].shape[1:])[c]
             for i, n in enumerate(self.out_names)}
            for c in range(N_CORES)
        ]


_CACHE = {}


def _get_runners():
    if "r" not in _CACHE:
        _CACHE["r"] = (_Runner(build_pass1(B_LOC)), _Runner(build_pass2(T_TILES)))
        # device-resident constant inputs (same every call)
        r2 = _CACHE["r"][1]
        wm_int = _build_WM()
        base = {
            "idn": np.eye(128, dtype=np.float32),
            "lw": _build_lw(),
            "pw": _build_pw(),
            "wm": wm_int,
            "wxa": _build_wxa(T_TILES),
            "wmf": None, "wml": None,
        }
        base["wmf"] = [_build_WM(first_tile=True)] + [wm_int] * (N_CORES - 1)
        base["wml"] = [wm_int] * (N_CORES - 1) + [_build_WM(last_tile=True)]
        dev = {}
        for name in ("idn", "lw", "pw", "wm", "wxa"):
            dev[name] = r2.put(np.concatenate([base[name]] * N_CORES, axis=0))
        for name in ("wmf", "wml"):
            dev[name] = r2.put(np.concatenate(base[name], axis=0))
        _CACHE["consts_dev"] = dev
    return _CACHE["r"]


def kernel(output, target):
    output = np.ascontiguousarray(np.asarray(output, dtype=np.float32))
    target = np.ascontiguousarray(np.asarray(target, dtype=np.float32))
    assert output.shape == (B_GLOB, PIX) and target.shape == (B_GLOB, PIX)

    run1, run2 = _get_runners()

    # ---- pass 1: minmax(target)  (concat over cores == full array)
    r1 = run1.run({"y": target})
    ymax = max(float(r["mm"][:, 0].max()) for r in r1)
    ymin = -max(float(r["mm"][:, 1].max()) for r in r1)
    dr = ymax - ymin
    C1 = (K1 * dr) ** 2
    C2 = (K2 * dr) ** 2

    # ---- pass 2
    zh = np.zeros((2, PIX), dtype=np.float32)
    xh = np.concatenate([output[(k + 1) * B_LOC:(k + 1) * B_LOC + 2]
                         if k < N_CORES - 1 else zh for k in range(N_CORES)], axis=0)
    yh = np.concatenate([target[(k + 1) * B_LOC:(k + 1) * B_LOC + 2]
                         if k < N_CORES - 1 else zh for k in range(N_CORES)], axis=0)
    cstrow = np.array([[C1 / 2, C2 / 2, C1, C2, 0, 0, 0, 0]], dtype=np.float32)
    ins = {
        "x": output, "y": target, "xh": xh, "yh": yh,
        "consts": np.concatenate([cstrow] * N_CORES, axis=0),
        **_CACHE["consts_dev"],
    }
    r2 = run2.run(ins)

    # ---- host combine (float64)
    tot = dict(mse_sum=0.0, ssim_sum=0.0, sxy=0.0, sxx=0.0, syy=0.0, sx=0.0, sy=0.0)
    for r in r2:
        cr = r["cross"].astype(np.float64)
        tot["mse_sum"] += 2.0 * r["mse_h"].astype(np.float64).sum()
        tot["ssim_sum"] += 4.0 * r["ssim_q"].astype(np.float64).sum()
        tot["sxy"] += r["gxy"].astype(np.float64).sum() + cr[:, 0].sum() + cr[:, 3].sum()
        tot["sxx"] += r["gxx"].astype(np.float64).sum() + cr[:, 1].sum()
        tot["syy"] += r["gyy"].astype(np.float64).sum() + cr[:, 2].sum()
        tot["sx"] += 4.0 * r["sx"].astype(np.float64).sum()
        tot["sy"] += 4.0 * r["sy"].astype(np.float64).sum()

    n = float(B_GLOB * PIX)
    mse = tot["mse_sum"] / n
    psnr = -10.0 * np.log10(mse)
    ssim_val = tot["ssim_sum"] / (B_GLOB * 324.0)
    cov = tot["sxy"] - tot["sx"] * tot["sy"] / n
    vx = tot["sxx"] - tot["sx"] ** 2 / n
    vy = tot["syy"] - tot["sy"] ** 2 / n
    epi = cov / np.sqrt(vx * vy)
    loss = MSE_W * mse + SSIM_W * (1.0 - ssim_val) + EPI_W * epi + PSNR_W * psnr
    return np.float32(loss)


# revision 3
# speedup vs baseline: 514.1764x; 419.8915x over previous
"""Trainium2 Bass kernel for CombinedMSESSIMLoss (MSE + SSIM + EPI + PSNR).

Contract: kernel(output, target) -> np.float32 scalar loss, computed on 8
NeuronCores, data-parallel over the batch dim (65536 images of 28x28).

Structure:
  pass 1 (per core): max(target), max(-target) over the 8192-image shard.
  host: combine -> data_range -> C1, C2 ssim constants.
  pass 2 (per core): all remaining partial sums in one streamed kernel:
    - transposes each 128-image tile to pixel-major via the PE,
    - SSIM 11x11 valid gaussian filtering as dense [112,108] PE matmuls over
      {x+y, x-y, x^2+y^2, 2xy} (error-compensated f32r weights),
    - ssim rational map on DVE/ACT/GPSIMD with per-tile accumulators,
    - per-image sobel d-maps as banded PE matmuls (exact integer weights),
    - batch-axis [1,2,1] smoothing handled algebraically: sum S_x S_y =
      sum_{|b-b'|<=2} M[b,b'] dx[b].dy[b'] with M = A A^T pentadiagonal,
      evaluated via PE gram matrices + weighted reductions; tile/core
      boundary pairs via stashed edge columns + one cross-gram at the end.
  host: assemble loss in float64 (log10, sqrt, divisions).
"""
import json

import numpy as np

import concourse.bass as bass
import concourse.tile as tile
from concourse import mybir

F32 = mybir.dt.float32
F32R = mybir.dt.float32r
ALU = mybir.AluOpType
ACTF = mybir.ActivationFunctionType
AX = mybir.AxisListType

H = W = 28
PIX = H * W
NCHUNK = 7
CK = 112
MOUT = 324
MCH = 3
MK = 108
WIN, SIGMA, K1, K2 = 11, 1.5, 0.01, 0.03
OW = 18
RS2 = float(1.0 / np.sqrt(2.0))

B_GLOB = 65536
N_CORES = 8
B_LOC = B_GLOB // N_CORES     # 8192
T_TILES = B_LOC // 128        # 64

MSE_W, SSIM_W, EPI_W, PSNR_W = 1.0, 0.5, 0.1, 0.01


# ---------------------------------------------------------------- walrus fix
# This walrus build rejects >1 sync-wait per instruction; split extra waits
# onto single-wait NoOps ahead of the instruction.
_orig_to_json_bytes = bass.Bass.to_json_bytes


def _split_waits(obj):
    if isinstance(obj, dict):
        ilist = obj.get("instructions")
        if isinstance(ilist, list):
            newlist = []
            for ins in ilist:
                try:
                    w = ins.get("sync_info", {}).get("on_wait", [])
                except AttributeError:
                    w = []
                if isinstance(w, list) and len(w) > 1:
                    for k, wt in enumerate(w[:-1]):
                        newlist.append({
                            "debug": ins.get("debug", 0),
                            "engine": ins["engine"],
                            "ins": [], "outs": [],
                            "name": str(ins["name"]) + f"_wsplit{k}",
                            "opcode": "NoOp",
                            "sync_info": {"on_update": [], "on_wait": [wt]},
                        })
                    ins["sync_info"]["on_wait"] = [w[-1]]
                newlist.append(ins)
            obj["instructions"] = newlist
        for v in obj.values():
            _split_waits(v)
    elif isinstance(obj, list):
        for v in obj:
            _split_waits(v)


def _patched_to_json_bytes(self, *a, **k):
    data = json.loads(_orig_to_json_bytes(self, *a, **k))
    _split_waits(data)
    return json.dumps(data).encode()


bass.Bass.to_json_bytes = _patched_to_json_bytes


# ----------------------------------------------------------- const builders

def _gauss1d():
    c = np.arange(WIN, dtype=np.float64) - WIN // 2
    g = np.exp(-(c ** 2) / (2.0 * SIGMA ** 2))
    return g / g.sum()


def _build_L():
    g = _gauss1d()
    L = np.zeros((PIX, MOUT), dtype=np.float64)
    for hp in range(OW):
        for wp in range(OW):
            q = hp * OW + wp
            for kh in range(WIN):
                for kw in range(WIN):
                    L[(hp + kh) * W + (wp + kw), q] += g[kh] * g[kw]
    return L


def _build_P():
    Sh = np.zeros((H, H))
    for hp in range(H):
        for dh, wgt in ((-1, 1.0), (0, 2.0), (1, 1.0)):
            Sh[min(max(hp + dh, 0), H - 1), hp] += wgt
    Dw = np.zeros((W, W))
    for wp in range(W):
        for dw, wgt in ((-1, -1.0), (1, 1.0)):
            Dw[min(max(wp + dw, 0), W - 1), wp] += wgt
    return np.einsum("ha,wb->hwab", Sh, Dw).reshape(PIX, PIX)


def _m_band(d):
    return {0: 6.0, 1: 4.0, 2: 1.0}.get(abs(d), 0.0)


def _build_WM(first_tile=False, last_tile=False):
    Wm = np.zeros((128, 128))
    for i in range(128):
        for j in range(max(0, i - 2), min(128, i + 3)):
            Wm[i, j] = _m_band(i - j)
    if first_tile:
        Wm[0, 0] = 10.0
        Wm[0, 1] = Wm[1, 0] = 5.0
    if last_tile:
        Wm[-1, -1] = 10.0
        Wm[-1, -2] = Wm[-2, -1] = 5.0
    return Wm.astype(np.float32)


def _build_wxa(T):
    Mc = np.array([[1.0, 0.0], [4.0, 1.0]])
    blk = np.zeros((2 * T, 2 * T))
    for g in range(T):
        blk[2 * g:2 * g + 2, 2 * g:2 * g + 2] = Mc
    return blk.astype(np.float32)


def _round_f32r(v):
    i = np.ascontiguousarray(v.astype(np.float32)).view(np.int32)
    r = ((i.astype(np.int64) + 0x800) & ~0xFFF).astype(np.int32)
    out = r.view(np.float32).copy()
    out[np.asarray(v) == 0] = 0.0
    return out


def _build_lw():
    L = _build_L()
    hi = _round_f32r(L)
    lo = _round_f32r(L - hi.astype(np.float64))
    lw = np.zeros((CK, 2, NCHUNK, MOUT), dtype=np.float32)
    for c in range(NCHUNK):
        lw[:, 0, c, :] = hi[c * CK:(c + 1) * CK, :]
        lw[:, 1, c, :] = lo[c * CK:(c + 1) * CK, :]
    return lw


def _build_pw():
    P = _build_P().astype(np.float32)
    pw = np.zeros((CK, NCHUNK, 3, CK), dtype=np.float32)
    for c in range(NCHUNK):
        for mr in range(3):
            m = c + mr - 1
            if 0 <= m < NCHUNK:
                pw[:, c, mr, :] = P[c * CK:(c + 1) * CK, m * CK:(m + 1) * CK]
    return pw


# ------------------------------------------------------------ kernel builders

def build_pass1(b_loc):
    nc = bass.Bass("TRN2", target_bir_lowering=False, debug=False, num_devices=1)
    y_d = nc.dram_tensor("y", [b_loc, PIX], F32, kind="ExternalInput")
    mm_d = nc.dram_tensor("mm", [128, 2], F32, kind="ExternalOutput")
    a = b_loc // 128
    yv = y_d.ap().rearrange("(p a) f -> p (a f)", p=128)
    CH = 8 if a % 8 == 0 else 1
    n_ch = a // CH
    FD = CH * PIX
    from contextlib import ExitStack
    with tile.TileContext(nc) as tc:
        with ExitStack() as ctx:
            io = ctx.enter_context(tc.tile_pool(name="io", bufs=3))
            wk = ctx.enter_context(tc.tile_pool(name="wk", bufs=2))
            accp = ctx.enter_context(tc.tile_pool(name="accp", bufs=1))
            mx = accp.tile([128, n_ch], F32)
            mn = accp.tile([128, n_ch], F32)
            for i in range(n_ch):
                t = io.tile([128, FD], F32, tag="t")
                nc.sync.dma_start(t[:], yv[:, i * FD:(i + 1) * FD])
                neg = wk.tile([128, FD], F32, tag="neg")
                nc.scalar.mul(neg[:], t[:], -1.0)
                nc.vector.tensor_reduce(mx[:, i:i + 1], t[:], AX.X, ALU.max)
                nc.vector.tensor_reduce(mn[:, i:i + 1], neg[:], AX.X, ALU.max)
            out = accp.tile([128, 2], F32)
            nc.vector.tensor_reduce(out[:, 0:1], mx[:], AX.X, ALU.max)
            nc.vector.tensor_reduce(out[:, 1:2], mn[:], AX.X, ALU.max)
            nc.sync.dma_start(mm_d.ap(), out[:])
    return nc


def build_pass2(T):
    from contextlib import ExitStack
    nc = bass.Bass("TRN2", target_bir_lowering=False, debug=False, num_devices=1)
    x_d = nc.dram_tensor("x", [T * 128, PIX], F32, kind="ExternalInput")
    y_d = nc.dram_tensor("y", [T * 128, PIX], F32, kind="ExternalInput")
    xh_d = nc.dram_tensor("xh", [2, PIX], F32, kind="ExternalInput")
    yh_d = nc.dram_tensor("yh", [2, PIX], F32, kind="ExternalInput")
    cst_d = nc.dram_tensor("consts", [1, 8], F32, kind="ExternalInput")
    idn_d = nc.dram_tensor("idn", [128, 128], F32, kind="ExternalInput")
    lw_d = nc.dram_tensor("lw", [CK, 2, NCHUNK, MOUT], F32R, kind="ExternalInput")
    pw_d = nc.dram_tensor("pw", [CK, NCHUNK, 3, CK], F32R, kind="ExternalInput")
    wm_d = nc.dram_tensor("wm", [128, 128], F32, kind="ExternalInput")
    wmf_d = nc.dram_tensor("wmf", [128, 128], F32, kind="ExternalInput")
    wml_d = nc.dram_tensor("wml", [128, 128], F32, kind="ExternalInput")
    wxa_d = nc.dram_tensor("wxa", [2 * T, 2 * T], F32, kind="ExternalInput")

    o_mse = nc.dram_tensor("mse_h", [128, T], F32, kind="ExternalOutput")
    o_ssim = nc.dram_tensor("ssim_q", [128, T], F32, kind="ExternalOutput")
    o_gxy = nc.dram_tensor("gxy", [128, T], F32, kind="ExternalOutput")
    o_gxx = nc.dram_tensor("gxx", [128, T], F32, kind="ExternalOutput")
    o_gyy = nc.dram_tensor("gyy", [128, T], F32, kind="ExternalOutput")
    o_sx = nc.dram_tensor("sx", [128, T], F32, kind="ExternalOutput")
    o_sy = nc.dram_tensor("sy", [128, T], F32, kind="ExternalOutput")
    o_cross = nc.dram_tensor("cross", [128, 4], F32, kind="ExternalOutput")

    xv = x_d.ap().rearrange("(t p) f -> t p f", p=128)
    yv = y_d.ap().rearrange("(t p) f -> t p f", p=128)

    with tile.TileContext(nc) as tc:
        with ExitStack() as ctx:
            const = ctx.enter_context(tc.tile_pool(name="const", bufs=1))
            io = ctx.enter_context(tc.tile_pool(name="io", bufs=3))
            wk = ctx.enter_context(tc.tile_pool(name="wk", bufs=2))
            mp = ctx.enter_context(tc.tile_pool(name="mp", bufs=2))
            ps = ctx.enter_context(tc.tile_pool(name="ps", bufs=1, space="PSUM"))
            accp = ctx.enter_context(tc.tile_pool(name="accp", bufs=1))
            stp = ctx.enter_context(tc.tile_pool(name="stp", bufs=1))

            cst = const.tile([128, 8], F32)
            _cap = cst_d.ap()
            nc.sync.dma_start(cst[:], bass.AP(tensor=_cap.tensor, offset=_cap.offset,
                                              ap=[[0, 128], [1, 8]]))
            c1h, c2h, C1s, C2s = (cst[:, i:i + 1] for i in range(4))

            idn = const.tile([128, 128], F32)
            nc.sync.dma_start(idn[:], idn_d.ap())
            lw = const.tile([CK, 2, NCHUNK, MOUT], F32R)
            nc.sync.dma_start(lw[:], lw_d.ap())
            pw = const.tile([CK, NCHUNK, 3, CK], F32R)
            nc.sync.dma_start(pw[:], pw_d.ap())
            wm = const.tile([128, 128], F32)
            nc.sync.dma_start(wm[:], wm_d.ap())
            wmf = const.tile([128, 128], F32)
            nc.sync.dma_start(wmf[:], wmf_d.ap())
            wml = const.tile([128, 128], F32)
            nc.sync.dma_start(wml[:], wml_d.ap())
            wxa = const.tile([2 * T, 2 * T], F32)
            nc.sync.dma_start(wxa[:], wxa_d.ap())

            a_mse = accp.tile([128, T], F32)
            a_ssim = accp.tile([128, T], F32)
            a_gxy = accp.tile([128, T], F32)
            a_gxx = accp.tile([128, T], F32)
            a_gyy = accp.tile([128, T], F32)
            a_sx = accp.tile([128, T], F32)
            a_sy = accp.tile([128, T], F32)
            a_cross = accp.tile([128, 4], F32)
            for a in (a_mse, a_ssim, a_gxy, a_gxx, a_gyy, a_sx, a_sy, a_cross):
                nc.vector.memset(a[:], 0.0)

            st_fx = stp.tile([CK, NCHUNK, T, 2], F32R)
            st_fy = stp.tile([CK, NCHUNK, T, 2], F32R)
            st_lx = stp.tile([CK, NCHUNK, T, 2], F32R)
            st_ly = stp.tile([CK, NCHUNK, T, 2], F32R)
            nc.vector.memset(st_fx[:].bitcast(F32), 0.0)
            nc.vector.memset(st_fy[:].bitcast(F32), 0.0)

            def process_tile(t, xs, ys, nb):
                is_halo = t == T
                tp = ps.tile([CK, 2, NCHUNK, 128], F32, tag="pA")
                for c in range(NCHUNK):
                    nc.tensor.transpose(tp[:, 0, c, 0:nb], xs[0:nb, c * CK:(c + 1) * CK], idn[0:nb, 0:nb])
                    nc.tensor.transpose(tp[:, 1, c, 0:nb], ys[0:nb, c * CK:(c + 1) * CK], idn[0:nb, 0:nb])
                rhsP = wk.tile([CK, NCHUNK, 2, 128], F32R, tag="rhsP")
                nc.scalar.copy(rhsP[:, :, 0, 0:nb], tp[:, 0, :, 0:nb])
                nc.scalar.copy(rhsP[:, :, 1, 0:nb], tp[:, 1, :, 0:nb])
                xT = rhsP[:, :, 0, :]
                yT = rhsP[:, :, 1, :]

                if not is_halo:
                    cb = wk.tile([CK, NCHUNK, 4, 128], F32R, tag="cb")
                    sT = cb[:, :, 0, :]
                    dT = cb[:, :, 1, :]
                    nc.gpsimd.tensor_add(sT, xT, yT)
                    nc.gpsimd.tensor_sub(dT, xT, yT)
                    s2 = wk.tile([CK, NCHUNK, 128], F32, tag="s2")
                    d2 = wk.tile([CK, NCHUNK, 128], F32, tag="d2")
                    nc.scalar.activation(s2[:], sT.bitcast(F32), ACTF.Square, bias=0.0, scale=RS2)
                    nc.scalar.activation(d2[:], dT.bitcast(F32), ACTF.Square, bias=0.0, scale=RS2,
                                         accum_out=a_mse[:CK, t:t + 1])
                    nc.vector.tensor_add(cb[:, :, 2, :], s2[:], d2[:])
                    nc.vector.tensor_sub(cb[:, :, 3, :], s2[:], d2[:])

                    mmL = ps.tile([MK, MCH, 512], F32, tag="pB")
                    for m in range(MCH):
                        for c in range(NCHUNK):
                            for hl in range(2):
                                nc.tensor.matmul(
                                    mmL[:, m, :], lw[:, hl, c, m * MK:(m + 1) * MK],
                                    cb[:, c, :, :].rearrange("p a b -> p (a b)"),
                                    start=(c == 0 and hl == 0),
                                    stop=(c == NCHUNK - 1 and hl == 1))

                    sqS = mp.tile([MK, MCH, 128], F32, tag="sqS")
                    sqD = mp.tile([MK, MCH, 128], F32, tag="sqD")
                    eSc = mp.tile([MK, MCH, 128], F32, tag="eSc")
                    e2c = mp.tile([MK, MCH, 128], F32, tag="e2c")
                    nc.scalar.activation(sqS[:], mmL[:, :, 0:128], ACTF.Square, bias=0.0, scale=0.5)
                    nc.scalar.activation(sqD[:], mmL[:, :, 128:256], ACTF.Square, bias=0.0, scale=0.5)
                    nc.scalar.activation(eSc[:], mmL[:, :, 256:384], ACTF.Identity, bias=C2s[:MK], scale=1.0)
                    nc.scalar.activation(e2c[:], mmL[:, :, 384:512], ACTF.Identity, bias=c2h[:MK], scale=0.5)

                    n1 = mp.tile([MK, MCH, 128], F32, tag="n1")
                    n2 = mp.tile([MK, MCH, 128], F32, tag="n2")
                    q = mp.tile([MK, MCH, 128], F32, tag="q")
                    d1 = mp.tile([MK, MCH, 128], F32, tag="d1")
                    dd2 = mp.tile([MK, MCH, 128], F32, tag="dd2")
                    num = mp.tile([MK, MCH, 128], F32, tag="num")
                    den = mp.tile([MK, MCH, 128], F32, tag="den")
                    rcp = mp.tile([MK, MCH, 128], F32, tag="rcp")
                    scr = mp.tile([MK, MCH, 128], F32, tag="scr")
                    nc.vector.scalar_tensor_tensor(n1[:], sqS[:], c1h[:MK], sqD[:], ALU.add, ALU.subtract)
                    nc.vector.scalar_tensor_tensor(n2[:], sqD[:], 0.0, sqS[:], ALU.add, ALU.subtract)
                    nc.vector.tensor_add(n2[:], n2[:], e2c[:])
                    nc.gpsimd.tensor_add(q[:], sqS[:], sqD[:])
                    nc.vector.tensor_scalar(d1[:], q[:], 2.0, C1s[:MK], ALU.mult, ALU.add)
                    nc.vector.scalar_tensor_tensor(dd2[:], q[:], -2.0, eSc[:], ALU.mult, ALU.add)
                    nc.vector.tensor_mul(num[:], n1[:], n2[:])
                    nc.gpsimd.tensor_mul(den[:], d1[:], dd2[:])
                    nc.vector.reciprocal(rcp[:], den[:])
                    nc.vector.scalar_tensor_tensor(scr[:], num[:], 1.0, rcp[:], ALU.mult, ALU.mult,
                                                   accum_out=a_ssim[:MK, t:t + 1])

                dP = ps.tile([CK, NCHUNK, 256], F32, tag="pA")
                nwid = 256 if nb == 128 else 2 * nb
                for m in range(NCHUNK):
                    cs = [c for c in range(NCHUNK) if abs(c - m) <= 1]
                    for k, c in enumerate(cs):
                        nc.tensor.matmul(
                            dP[:, m, 0:nwid], pw[:, c, m - c + 1, :],
                            rhsP[:, c, :, 0:nb],
                            start=(k == 0), stop=(k == len(cs) - 1))

                if not is_halo:
                    rhsG = wk.tile([CK, NCHUNK, 258], F32R, tag="rhsG")
                    nc.scalar.copy(rhsG[:, :, 0:128], dP[:, :, 128:256])
                    nc.scalar.copy(rhsG[:, :, 128:256], dP[:, :, 0:128])
                    nc.vector.memset(rhsG[:, :, 256:257].bitcast(F32), 1.0)
                    nc.vector.memset(rhsG[:, :, 257:258].bitcast(F32), 0.0)
                    dyT = rhsG[:, :, 0:128]
                    dxT = rhsG[:, :, 128:256]
                    nc.vector.tensor_copy(st_lx[:, :, t, :], dxT[:, :, 126:128])
                    nc.vector.tensor_copy(st_ly[:, :, t, :], dyT[:, :, 126:128])
                    if t > 0:
                        nc.vector.tensor_copy(st_fx[:, :, t - 1, :], dxT[:, :, 0:2])
                        nc.vector.tensor_copy(st_fy[:, :, t - 1, :], dyT[:, :, 0:2])
                    gg = ps.tile([128, 2, 512], F32, tag="pB")
                    for c in range(NCHUNK):
                        nc.tensor.matmul(gg[:, 0, 0:258], dxT[:, c, :], rhsG[:, c, :],
                                         start=(c == 0), stop=(c == NCHUNK - 1))
                    for c in range(NCHUNK):
                        nc.tensor.matmul(gg[:, 1, 0:258], dyT[:, c, :], rhsG[:, c, :],
                                         start=(c == 0), stop=(c == NCHUNK - 1))
                    wsel = wmf if t == 0 else (wml if t == T - 1 else wm)
                    gs = mp.tile([128, 3, 128], F32, tag="gs")
                    nc.vector.scalar_tensor_tensor(gs[:, 0, :], gg[:, 0, 0:128], 1.0, wsel[:],
                                                   ALU.mult, ALU.mult, accum_out=a_gxy[:, t:t + 1])
                    nc.vector.scalar_tensor_tensor(gs[:, 1, :], gg[:, 0, 128:256], 1.0, wsel[:],
                                                   ALU.mult, ALU.mult, accum_out=a_gxx[:, t:t + 1])
                    nc.vector.scalar_tensor_tensor(gs[:, 2, :], gg[:, 1, 0:128], 1.0, wsel[:],
                                                   ALU.mult, ALU.mult, accum_out=a_gyy[:, t:t + 1])
                    nc.vector.tensor_copy(a_sx[:, t:t + 1], gg[:, 0, 256:257])
                    nc.vector.tensor_copy(a_sy[:, t:t + 1], gg[:, 1, 256:257])
                else:
                    hd = wk.tile([CK, NCHUNK, 4], F32R, tag="hd")
                    nc.scalar.copy(hd[:, :, 0:2], dP[:, :, 2:4])
                    nc.scalar.copy(hd[:, :, 2:4], dP[:, :, 0:2])
                    nc.vector.tensor_copy(st_fy[:, :, T - 1, :], hd[:, :, 0:2])
                    nc.vector.tensor_copy(st_fx[:, :, T - 1, :], hd[:, :, 2:4])

            for t in range(T):
                xs = io.tile([128, PIX], F32, tag="xs")
                ys = io.tile([128, PIX], F32, tag="ys")
                nc.sync.dma_start(xs[:], xv[t])
                nc.sync.dma_start(ys[:], yv[t])
                process_tile(t, xs, ys, 128)

            xs = io.tile([128, PIX], F32, tag="xs")
            ys = io.tile([128, PIX], F32, tag="ys")
            nc.vector.memset(xs[:], 0.0)
            nc.vector.memset(ys[:], 0.0)
            nc.sync.dma_start(xs[0:2, :], xh_d.ap())
            nc.sync.dma_start(ys[0:2, :], yh_d.ap())
            process_tile(T, xs, ys, 2)

            n2t = 2 * T
            sfx = st_fx[:].rearrange("p c t i -> p c (t i)")
            sfy = st_fy[:].rearrange("p c t i -> p c (t i)")
            slx = st_lx[:].rearrange("p c t i -> p c (t i)")
            sly = st_ly[:].rearrange("p c t i -> p c (t i)")
            rhsX = wk.tile([CK, NCHUNK, 2 * n2t], F32R, tag="rhsX")
            nc.vector.tensor_copy(rhsX[:, :, 0:n2t], sfy)
            nc.vector.tensor_copy(rhsX[:, :, n2t:2 * n2t], sfx)
            gX = ps.tile([n2t, 2, 2 * n2t], F32, tag="pB")
            for c in range(NCHUNK):
                nc.tensor.matmul(gX[:, 0, :], slx[:, c, :], rhsX[:, c, :],
                                 start=(c == 0), stop=(c == NCHUNK - 1))
            for c in range(NCHUNK):
                nc.tensor.matmul(gX[:, 1, :], sly[:, c, :], rhsX[:, c, :],
                                 start=(c == 0), stop=(c == NCHUNK - 1))
            xscr = mp.tile([n2t, 4, n2t], F32, tag="xscr")
            nc.vector.scalar_tensor_tensor(xscr[:, 0, :], gX[:, 0, 0:n2t], 1.0, wxa[:],
                                           ALU.mult, ALU.mult, accum_out=a_cross[0:n2t, 0:1])
            nc.vector.scalar_tensor_tensor(xscr[:, 1, :], gX[:, 0, n2t:2 * n2t], 2.0, wxa[:],
                                           ALU.mult, ALU.mult, accum_out=a_cross[0:n2t, 1:2])
            nc.vector.scalar_tensor_tensor(xscr[:, 2, :], gX[:, 1, 0:n2t], 2.0, wxa[:],
                                           ALU.mult, ALU.mult, accum_out=a_cross[0:n2t, 2:3])
            nc.vector.scalar_tensor_tensor(xscr[:, 3, :], gX[:, 1, n2t:2 * n2t], 1.0, wxa[:],
                                           ALU.mult, ALU.mult, accum_out=a_cross[0:n2t, 3:4])

            nc.sync.dma_start(o_mse.ap(), a_mse[:])
            nc.sync.dma_start(o_ssim.ap(), a_ssim[:])
            nc.sync.dma_start(o_gxy.ap(), a_gxy[:])
            nc.sync.dma_start(o_gxx.ap(), a_gxx[:])
            nc.sync.dma_start(o_gyy.ap(), a_gyy[:])
            nc.sync.dma_start(o_sx.ap(), a_sx[:])
            nc.sync.dma_start(o_sy.ap(), a_sy[:])
            nc.sync.dma_start(o_cross.ap(), a_cross[:])
    return nc


# ---------------------------------------------------------------- driver


class _Runner:
    """Caches the shard_map-jitted executable for a built Bass module."""

    def __init__(self, nc):
        import jax
        from jax.sharding import Mesh, PartitionSpec
        from jax.experimental.shard_map import shard_map
        from concourse.bass2jax import (_bass_exec_p, install_neuronx_cc_hook,
                                        partition_id_tensor)
        install_neuronx_cc_hook()
        self.jax = jax
        partition_name = (nc.partition_id_tensor.name
                          if nc.partition_id_tensor else None)
        in_names, out_names, out_avals, zero_outs = [], [], [], []
        for alloc in nc.m.functions[0].allocations:
            if not isinstance(alloc, mybir.MemoryLocationSet):
                continue
            name = alloc.memorylocations[0].name
            if alloc.kind == "ExternalInput":
                if name != partition_name:
                    in_names.append(name)
            elif alloc.kind == "ExternalOutput":
                out_names.append(name)
                shape = tuple(alloc.tensor_shape)
                dtype = mybir.dt.np(alloc.dtype)
                out_avals.append(jax.core.ShapedArray(shape, dtype))
                zero_outs.append(np.zeros(shape, dtype))
        self.in_names = in_names
        self.out_names = out_names
        self.out_avals = out_avals
        n_params = len(in_names)
        n_outs = len(out_avals)
        all_in = list(in_names) + list(out_names)
        if partition_name is not None:
            all_in.append(partition_name)

        def _body(*args):
            operands = list(args)
            if partition_name is not None:
                operands.append(partition_id_tensor())
            return tuple(_bass_exec_p.bind(
                *operands, out_avals=tuple(out_avals), in_names=tuple(all_in),
                out_names=tuple(out_names), lowering_input_output_aliases=(),
                sim_require_finite=True, sim_require_nnan=True, nc=nc))

        devices = jax.devices()[:N_CORES]
        self.mesh = Mesh(np.asarray(devices), ("core",))
        self.sharding = jax.sharding.NamedSharding(self.mesh, PartitionSpec("core"))
        in_specs = (PartitionSpec("core"),) * (n_params + n_outs)
        out_specs = (PartitionSpec("core"),) * n_outs
        self.fn = jax.jit(
            shard_map(_body, mesh=self.mesh, in_specs=in_specs,
                      out_specs=out_specs, check_rep=False),
            keep_unused=True)
        self.zero_dev = [
            jax.device_put(np.zeros((N_CORES * z.shape[0],) + z.shape[1:], z.dtype),
                           self.sharding) for z in zero_outs]

    def put(self, arr):
        return self.jax.device_put(arr, self.sharding)

    def run(self, concat_inputs):
        args = [concat_inputs[n] if not isinstance(concat_inputs[n], np.ndarray)
                else self.put(concat_inputs[n]) for n in self.in_names]
        outs = self.fn(*args, *self.zero_dev)
        outs = [np.asarray(o) for o in outs]
        return [
            {n: outs[i].reshape((N_CORES, outs[i].shape[0] // N_CORES)
                                + outs[i].shape[1:])[c]
             for i, n in enumerate(self.out_names)}
            for c in range(N_CORES)
        ]


_CACHE = {}


def _get_runners():
    if "r" not in _CACHE:
        _CACHE["r"] = (_Runner(build_pass1(B_LOC)), _Runner(build_pass2(T_TILES)))
        # device-resident constant inputs (same every call)
        r2 = _CACHE["r"][1]
        wm_int = _build_WM()
        base = {
            "idn": np.eye(128, dtype=np.float32),
            "lw": _build_lw(),
            "pw": _build_pw(),
            "wm": wm_int,
            "wxa": _build_wxa(T_TILES),
            "wmf": None, "wml": None,
        }
        base["wmf"] = [_build_WM(first_tile=True)] + [wm_int] * (N_CORES - 1)
        base["wml"] = [wm_int] * (N_CORES - 1) + [_build_WM(last_tile=True)]
        dev = {}
        for name in ("idn", "lw", "pw", "wm", "wxa"):
            dev[name] = r2.put(np.concatenate([base[name]] * N_CORES, axis=0))
        for name in ("wmf", "wml"):
            dev[name] = r2.put(np.concatenate(base[name], axis=0))
        _CACHE["consts_dev"] = dev
    return _CACHE["r"]


def kernel(output, target):
    output = np.ascontiguousarray(np.asarray(output, dtype=np.float32))
    target = np.ascontiguousarray(np.asarray(target, dtype=np.float32))
    assert output.shape == (B_GLOB, PIX) and target.shape == (B_GLOB, PIX)

    run1, run2 = _get_runners()

    # ---- pass 1: minmax(target)  (concat over cores == full array)
    # device_put x and y up front; y is shared by both passes.
    y_dev = run1.put(target)
    x_dev = run2.put(output)
    r1 = run1.run({"y": y_dev})
    ymax = max(float(r["mm"][:, 0].max()) for r in r1)
    ymin = -max(float(r["mm"][:, 1].max()) for r in r1)
    dr = ymax - ymin
    C1 = (K1 * dr) ** 2
    C2 = (K2 * dr) ** 2

    # ---- pass 2
    zh = np.zeros((2, PIX), dtype=np.float32)
    xh = np.concatenate([output[(k + 1) * B_LOC:(k + 1) * B_LOC + 2]
                         if k < N_CORES - 1 else zh for k in range(N_CORES)], axis=0)
    yh = np.concatenate([target[(k + 1) * B_LOC:(k + 1) * B_LOC + 2]
                         if k < N_CORES - 1 else zh for k in range(N_CORES)], axis=0)
    cstrow = np.array([[C1 / 2, C2 / 2, C1, C2, 0, 0, 0, 0]], dtype=np.float32)
    ins = {
        "x": x_dev, "y": y_dev, "xh": xh, "yh": yh,
        "consts": np.concatenate([cstrow] * N_CORES, axis=0),
        **_CACHE["consts_dev"],
    }
    r2 = run2.run(ins)

    # ---- host combine (float64)
    tot = dict(mse_sum=0.0, ssim_sum=0.0, sxy=0.0, sxx=0.0, syy=0.0, sx=0.0, sy=0.0)
    for r in r2:
        cr = r["cross"].astype(np.float64)
        tot["mse_sum"] += 2.0 * r["mse_h"].astype(np.float64).sum()
        tot["ssim_sum"] += 4.0 * r["ssim_q"].astype(np.float64).sum()
        tot["sxy"] += r["gxy"].astype(np.float64).sum() + cr[:, 0].sum() + cr[:, 3].sum()
        tot["sxx"] += r["gxx"].astype(np.float64).sum() + cr[:, 1].sum()
        tot["syy"] += r["gyy"].astype(np.float64).sum() + cr[:, 2].sum()
        tot["sx"] += 4.0 * r["sx"].astype(np.float64).sum()
        tot["sy"] += 4.0 * r["sy"].astype(np.float64).sum()

    n = float(B_GLOB * PIX)
    mse = tot["mse_sum"] / n
    psnr = -10.0 * np.log10(mse)
    ssim_val = tot["ssim_sum"] / (B_GLOB * 324.0)
    cov = tot["sxy"] - tot["sx"] * tot["sy"] / n
    vx = tot["sxx"] - tot["sx"] ** 2 / n
    vy = tot["syy"] - tot["sy"] ** 2 / n
    epi = cov / np.sqrt(vx * vy)
    loss = MSE_W * mse + SSIM_W * (1.0 - ssim_val) + EPI_W * epi + PSNR_W * psnr
    return np.float32(loss)
